# revision 63
# baseline (speedup 1.0000x reference)
"""Trainium2 Bass kernel for nn_AttentionBlock_32238024524154.

A 2-layer cross-attention transformer block (ref returns (c, q)):
    q  = LN(query)
    c  = MHA1(q, LN(context))            # no residual
    c  = c + MLP1(LN(c))
    c  = c + MHA2(q, LN(c))
    c  = c + MLP2(LN(c))

Sharding: data-parallel over batch — 8 batch elements -> 8 NeuronCores,
one element per core, no collectives.

Per-core kernel strategy:
  - Activations feature-major ("X^T": [feature, token]) in SBUF between
    matmuls; bf16 everywhere the PE reads (weights converted on host, LN
    outputs, residual stream, probs, V, hidden).  PSUM accumulates fp32.
    bf16 halves the PE transpose cost (cost tracks the moving operand's
    dtype), halves weight DMA, and doubles DVE throughput on SBUF ops.
  - ln_w/ln_b folded into consumer weights ON THE HOST (exact).  The k-bias
    is dropped (softmax shift invariance); the v-bias is folded into the
    output-projection bias (softmax rows sum to 1), both exact.
  - LayerNorm: token-major bn_stats on DVE, sqrt on ACT (table prewarmed at
    t=0), fast-approx reciprocal on DVE, normalize-apply on ACT as
    Identity(x*rstd - mu*rstd) with per-partition scale/bias APs.  PSUM
    copies are parity-split across ACT and DVE.
  - Emission order feeds the PE from the first microsecond: context LN +
    K/V projection first (first key block split 2+2 tiles), query LN +
    both Q projections second; const bias vectors are emitted after the
    x-tile DMAs so they never head the SP DMA queue.
  - Attention scores transposed (S^T = [keys, queries]); the two heads of a
    chunk interleave per key tile on disjoint PE row groups (base_partition
    0/64) so hardware can overlap the half-array matmuls; one exp per
    [128,1024] PSUM tile covers both heads; softmax row-sums ride a
    ones-column in V; normalize via fast reciprocal + GPSIMD partition
    broadcast.  No max-subtraction (scores are O(1) by construction).
  - Each finished 512-query half of the attention projection / MLP output
    immediately feeds the next LayerNorm (on_half callbacks), overlapping
    LN phases with the producing phase's second half.
  - MLP hidden in 3 blocks of 1024; residual + bias pre-folded into the
    accumulator at block 0; the last MLP streams each finished half of the
    final output straight to DRAM (transpose + copy + DMA).
  - Big weight streams ride the gpsimd SWDGE / SP queues chosen per phase
    so they never queue behind latency-critical activation DMAs.
"""

import numpy as np
from contextlib import ExitStack

try:
    import concourse.bass as bass
except ImportError:  # pragma: no cover
    import sys

    sys.path.insert(0, "/opt/trn_rl_repo")
    import concourse.bass as bass

import ml_dtypes
import concourse.bacc as bacc
import concourse.tile as tile
from concourse import mybir
from concourse.bass_utils import run_bass_kernel_spmd
from concourse.masks import make_identity

F32 = mybir.dt.float32
BF16 = mybir.dt.bfloat16
AF = mybir.ActivationFunctionType
ALU = mybir.AluOpType

P = 128
D = 768            # model dim
FC = D // P        # 6 feature chunks
DIM = 384          # attention inner dim
QC = DIM // P      # 3 chunks of q/k features
H = 6              # heads
DH = 64            # head dim
NQ = 1024          # query tokens per batch element
NQT = NQ // P      # 8
NK1 = 2048         # context tokens
HID = 3072
HB = 1024          # mlp hidden block
EPS = 1e-5
SCALE = DH ** -0.5
N_CORES = 8


def _emit(nc, tc, ctx, io):
    MUL, ADD = ALU.mult, ALU.add

    # ---------------- constants ----------------
    consts = ctx.enter_context(tc.tile_pool(name="consts", bufs=1))

    ident = consts.tile([P, P], BF16)
    make_identity(nc, ident[:])

    epsb = consts.tile([P, 1], F32)
    nc.vector.memset(epsb[:], EPS)
    warmp = ctx.enter_context(tc.tile_pool(name="warm", bufs=2))

    def warm_table(func):
        # tiny activation with no data deps: hoists the ACT table load off
        # the critical path of the first real op needing `func`
        w = warmp.tile([1, 1], F32, tag="warm")
        nc.scalar.activation(w[:], epsb[0:1, :], func)

    # prewarm the Sqrt/Identity ACT table during the initial DMA wait
    warm_table(AF.Sqrt)

    def feat_major_vec(name, n):
        t = consts.tile([P, n // P], F32, tag=f"fmv_{name}")
        nc.sync.dma_start(t[:], io[name].rearrange("(c p) -> p c", p=P))
        return t


    # ---------------- shared working pools ----------------
    xtok = ctx.enter_context(tc.tile_pool(name="xtok", bufs=4))
    xhatp = ctx.enter_context(tc.tile_pool(name="xhatp", bufs=6))
    stats = ctx.enter_context(tc.tile_pool(name="stats", bufs=4))

    def ln_run(src, tiles, dst_fT, psum, q_out=None, qo_pool=None,
               dst_col_off=None, dma_eng=None):
        """LayerNorm-normalize len(tiles) 128-token tiles into a feature-major
        bf16 destination.  src: ("dram", ap) or ("feat", tile).  Tile t lands
        at dst cols dst_col_off + (t - tiles[0])*P (dst_col_off defaults 0)."""
        if dst_col_off is None:
            dst_col_off = 0
        kind, sap = src
        G = len(tiles)
        assert G <= 8
        mvb = stats.tile([P, 8, 2], F32, tag="mvb")
        xs = []
        for i, t in enumerate(tiles):
            if kind == "dram":
                x = xtok.tile([P, D], F32, tag="xtok", bufs=6)
                eng = (dma_eng[i] if isinstance(dma_eng, list)
                       else (dma_eng or nc.sync))
                eng.dma_start(x[:], sap[t * P:(t + 1) * P, :])
            else:
                p0 = psum.tile([P, 512], BF16, tag="lnp")
                for c in range(4):
                    nc.tensor.transpose(p0[:, c * P:(c + 1) * P],
                                        sap[:, c, t * P:(t + 1) * P], ident[:])
                p1 = psum.tile([P, 512], BF16, tag="lnp")
                for c in range(2):
                    nc.tensor.transpose(p1[:, c * P:(c + 1) * P],
                                        sap[:, 4 + c, t * P:(t + 1) * P], ident[:])
                x = xtok.tile([P, D], BF16, tag="xtokb", bufs=9)
                if t % 2 == 0:
                    nc.scalar.copy(x[:, 0:512], p0[:])
                    nc.vector.tensor_copy(x[:, 512:D], p1[:, 0:256])
                else:
                    nc.vector.tensor_copy(x[:, 0:512], p0[:])
                    nc.scalar.copy(x[:, 512:D], p1[:, 0:256])
            xs.append(x)
            st = stats.tile([P, 2, 6], F32, tag="bnst")
            nc.vector.bn_stats(st[:, 0, :], x[:, 0:512])
            nc.vector.bn_stats(st[:, 1, :], x[:, 512:D])
            nc.vector.bn_aggr(mvb[:, i, :], st[:])
        sd = stats.tile([P, 8], F32, tag="sd")
        nc.scalar.activation(sd[:, :G], mvb[:, 0:G, 1], AF.Sqrt, bias=epsb[:])
        rstd = stats.tile([P, 8], F32, tag="rstd")
        nc.vector.reciprocal_approx_fast(rstd[:, :G], sd[:, :G])
        nmr = stats.tile([P, 8], F32, tag="nmr")
        nc.vector.scalar_tensor_tensor(nmr[:, :G], mvb[:, 0:G, 0], -1.0,
                                       rstd[:, :G], op0=MUL, op1=MUL)
        for i, t in enumerate(tiles):
            xh = xhatp.tile([P, D], BF16, tag="xhat")
            nc.scalar.activation(xh[:], xs[i][:], AF.Identity,
                                 bias=nmr[:, i:i + 1], scale=rstd[:, i:i + 1])
            if q_out is not None:
                qo = qo_pool.tile([P, D], F32, tag="qo")
                nc.vector.tensor_mul(qo[:], xh[:], wbc[:])
                nc.gpsimd.tensor_add(qo[:], qo[:], bbc[:])
                nc.sync.dma_start(q_out[t * P:(t + 1) * P, :], qo[:])
            col = dst_col_off + (t - tiles[0]) * P
            o0 = psum.tile([P, 512], BF16, tag="lnp")
            for c in range(4):
                nc.tensor.transpose(o0[:, c * P:(c + 1) * P],
                                    xh[:, c * P:(c + 1) * P], ident[:])
            o1 = psum.tile([P, 512], BF16, tag="lnp")
            for c in range(2):
                nc.tensor.transpose(o1[:, c * P:(c + 1) * P],
                                    xh[:, (4 + c) * P:(5 + c) * P], ident[:])
            if t % 2 == 0:
                nc.vector.tensor_copy(dst_fT[:, 0:4, col:col + P],
                                      o0[:].rearrange("p (c t) -> p c t", c=4))
                nc.scalar.copy(dst_fT[:, 4:6, col:col + P],
                               o1[:, 0:256].rearrange("p (c t) -> p c t", c=2))
            else:
                nc.scalar.copy(dst_fT[:, 0:4, col:col + P],
                               o0[:].rearrange("p (c t) -> p c t", c=4))
                nc.vector.tensor_copy(dst_fT[:, 4:6, col:col + P],
                                      o1[:, 0:256].rearrange("p (c t) -> p c t", c=2))

    # ---------------- persistent activation pools ----------------
    qTp_pool = tc.alloc_tile_pool(name="qTp", bufs=1, side="right")
    qTp2_pool = ctx.enter_context(tc.tile_pool(name="qTp2", bufs=1))
    cstream = ctx.enter_context(tc.tile_pool(name="cstream", bufs=2))
    lnct_pool = ctx.enter_context(tc.tile_pool(name="lnct", bufs=1))
    pT_pool = ctx.enter_context(tc.tile_pool(name="pT", bufs=5))
    small = ctx.enter_context(tc.tile_pool(name="small", bufs=4))

    # ---------------- K/V projection for one 512-key block ----------------
    def kv_v_unit(cnap, c0, kb, k4, wkv, v, pskv, act_copies=True):
        ps = pskv.tile([P, 512], F32, tag="pskv")
        for f in range(FC):
            nc.tensor.matmul(
                ps[:, 0:DIM],
                cnap[:, f, c0 + k4 * P:c0 + (k4 + 1) * P],
                wkv[:, f, DIM:2 * DIM],
                start=(f == 0), stop=(f == FC - 1))
        if act_copies:
            nc.scalar.copy(v[:, kb * 4 + k4, :, 0:DH],
                           ps[:, 0:DIM].rearrange("p (h d) -> p h d", h=H))
        else:
            nc.vector.tensor_copy(v[:, kb * 4 + k4, :, 0:DH],
                                  ps[:, 0:DIM].rearrange("p (h d) -> p h d", h=H))

    def kv_k_unit(cnap, c0, kb, qc, wkv, kT, pskv, act_copies=True):
        ps = pskv.tile([P, 512], F32, tag="pskv")
        for f in range(FC):
            nc.tensor.matmul(
                ps[:], wkv[:, f, qc * P:(qc + 1) * P],
                cnap[:, f, c0:c0 + 512],
                start=(f == 0), stop=(f == FC - 1))
        if act_copies:
            nc.scalar.copy(kT[:, qc, kb * 512:(kb + 1) * 512], ps[:])
        else:
            nc.vector.tensor_copy(kT[:, qc, kb * 512:(kb + 1) * 512], ps[:])

    def kv_block(cnap, c0, kb, wkv, kT, v, pskv, act_copies=True):
        for k4 in range(4):
            kv_v_unit(cnap, c0, kb, k4, wkv, v, pskv, act_copies)
        for qc in range(QC):
            kv_k_unit(cnap, c0, kb, qc, wkv, kT, pskv, act_copies)


    # ---------------- attention + output projection -----------------------
    def mha_attn(kT, v, m, qTp, wp_name, bpf, residual, dst, on_half=None,
                 kv_tail=None):
        mt = m // P
        with tc.tile_pool(name="psA", bufs=2 if kv_tail else 4,
                          space="PSUM") as psA_pool, \
             tc.tile_pool(name="outT", bufs=1) as outT_pool:
            outT = outT_pool.tile([P, QC, NQ], BF16, tag="outT")
            with ExitStack() as ses:
                psS_pool = ses.enter_context(
                    tc.tile_pool(name="psS", bufs=2, space="PSUM"))
                pskvt = None
                units = []
                if kv_tail is not None:
                    pskvt = ses.enter_context(
                        tc.tile_pool(name="pskvt", bufs=2, space="PSUM"))
                    units = list(kv_tail)
                ui = [0]

                def pop_unit():
                    if ui[0] < len(units):
                        units[ui[0]](pskvt)
                        ui[0] += 1

                for qh in range(2):
                    for kc in range(QC):
                        psA = [psA_pool.tile([P, 512], F32, tag="psA",
                                             name=f"psA_{qh}_{kc}_{hh}")
                               for hh in range(2)]
                        pTs = {}

                        def do_S(kt):
                            psS = psS_pool.tile([P, 1024], F32, tag="psS")
                            for hh in range(2):
                                nc.tensor.matmul(
                                    psS[:, hh * 512:(hh + 1) * 512],
                                    kT[hh * DH:(hh + 1) * DH, kc, kt * P:(kt + 1) * P],
                                    qTp[hh * DH:(hh + 1) * DH, kc, qh * 512:(qh + 1) * 512],
                                    start=True, stop=True)
                            pT = pT_pool.tile([P, 1024], BF16, tag="pT")
                            nc.scalar.activation(pT[:], psS[:], AF.Exp)
                            pTs[kt] = pT

                        def do_av(kt):
                            pT = pTs.pop(kt)
                            for hh in range(2):
                                nc.tensor.matmul(
                                    psA[hh][0:DH + 1, :], v[:, kt, 2 * kc + hh, :],
                                    pT[:, hh * 512:(hh + 1) * 512],
                                    start=(kt == 0), stop=(kt == mt - 1))

                        for kt in range(mt):
                            do_S(kt)
                            pop_unit()
                            if kt >= 2:
                                do_av(kt - 2)
                        do_av(mt - 2)
                        do_av(mt - 1)
                        for hh in range(2):
                            rowsum = small.tile([1, 512], F32, tag="rowsum")
                            nc.vector.tensor_copy(rowsum[:], psA[hh][DH:DH + 1, :])
                            rcp = small.tile([1, 512], F32, tag="rcp")
                            nc.vector.reciprocal_approx_fast(rcp[:], rowsum[:])
                            rbs = small.tile([DH, 512], F32, tag="rbs", bufs=2)
                            nc.gpsimd.partition_broadcast(rbs[:], rcp[:])
                            nc.vector.tensor_mul(
                                outT[hh * DH:(hh + 1) * DH, kc, qh * 512:(qh + 1) * 512],
                                psA[hh][0:DH, :], rbs[:])

            # output projection (+bias, +optional residual); qh-outer so the
            # follow-up LN of each finished half (on_half) overlaps the rest
            with tc.tile_pool(name="wp", bufs=1) as wp_pool, \
                 tc.tile_pool(name="lnpa", bufs=2, space="PSUM") as lnp:
                wp = wp_pool.tile([P, QC, D], BF16, tag="wp")
                nc.sync.dma_start(wp[:], io[wp_name].rearrange("(c p) d -> p c d", p=P))
                for qh in range(2):
                    for mc in range(FC):
                        ps = psA_pool.tile([P, 512], F32, tag="psA")
                        for qc in range(QC):
                            nc.tensor.matmul(
                                ps[:], wp[:, qc, mc * P:(mc + 1) * P],
                                outT[:, qc, qh * 512:(qh + 1) * 512],
                                start=(qc == 0), stop=(qc == QC - 1))
                        dsl = dst[:, mc, qh * 512:(qh + 1) * 512]
                        if residual is None:
                            nc.vector.tensor_scalar(
                                dsl, ps[:], bpf[:, mc:mc + 1], None, op0=ADD)
                        else:
                            nc.vector.scalar_tensor_tensor(
                                dsl, ps[:], bpf[:, mc:mc + 1],
                                residual[:, mc, qh * 512:(qh + 1) * 512],
                                op0=ADD, op1=ADD)
                    if on_half is not None:
                        on_half(qh, lnp)

    # ---------------- MLP ----------------
    def mlp(lnsrc, w1_name, b1f, w2_name, b2f, residual, dst, on_half=None):
        nhb = HID // HB
        hm_n = HB // P
        with tc.tile_pool(name="w1", bufs=2) as w1_pool, \
             tc.tile_pool(name="w2", bufs=2) as w2_pool, \
             tc.tile_pool(name="hT", bufs=2) as h_pool, \
             tc.tile_pool(name="acc", bufs=2) as acc_pool, \
             tc.tile_pool(name="psf1", bufs=3, space="PSUM") as psf1, \
             tc.tile_pool(name="psf2", bufs=3, space="PSUM") as psf2, \
             tc.tile_pool(name="lnpm", bufs=2, space="PSUM") as lnp:
            accs = [acc_pool.tile([P, FC, 512], BF16, tag="acc", name=f"acc{i}")
                    for i in range(2)]
            w1re = io[w1_name].rearrange("(c p) h -> p c h", p=P)
            w2re = io[w2_name].rearrange("(b p) d -> p b d", p=P)
            for hb in range(nhb):
                w1b = w1_pool.tile([P, FC, HB], BF16, tag="w1b")
                nc.sync.dma_start(w1b[:], w1re[:, :, hb * HB:(hb + 1) * HB])
                w2b = w2_pool.tile([P, hm_n, D], BF16, tag="w2b")
                nc.sync.dma_start(w2b[:], w2re[:, hb * hm_n:(hb + 1) * hm_n, :])
                for qh in range(2):
                    hT = h_pool.tile([P, hm_n, 512], BF16, tag="hT")
                    for hm in range(hm_n):
                        hk = hb * hm_n + hm
                        ps = psf1.tile([P, 512], F32, tag="psf1")
                        for f in range(FC):
                            nc.tensor.matmul(
                                ps[:], w1b[:, f, hm * P:(hm + 1) * P],
                                lnsrc[:, f, qh * 512:(qh + 1) * 512],
                                start=(f == 0), stop=(f == FC - 1))
                        nc.scalar.activation(hT[:, hm, :], ps[:], AF.Gelu,
                                             bias=b1f[:, hk:hk + 1])
                    for mc in range(FC):
                        ps2 = psf2.tile([P, 512], F32, tag="psf2")
                        for hm in range(hm_n):
                            nc.tensor.matmul(
                                ps2[:], w2b[:, hm, mc * P:(mc + 1) * P],
                                hT[:, hm, :],
                                start=(hm == 0), stop=(hm == hm_n - 1))
                        if hb == 0:
                            nc.vector.scalar_tensor_tensor(
                                accs[qh][:, mc, :], ps2[:], b2f[:, mc:mc + 1],
                                residual[:, mc, qh * 512:(qh + 1) * 512],
                                op0=ADD, op1=ADD)
                        elif hb < nhb - 1:
                            nc.vector.tensor_add(accs[qh][:, mc, :],
                                                 accs[qh][:, mc, :], ps2[:])
                        else:
                            nc.vector.tensor_add(
                                dst[:, mc, qh * 512:(qh + 1) * 512],
                                accs[qh][:, mc, :], ps2[:])
                    if hb == nhb - 1 and on_half is not None:
                        on_half(qh, lnp)

    # ======================= block body =======================
    # ---- phase A: LN(context) + all four K/V projection blocks
    kv1_es = ExitStack()
    kv1_pool = kv1_es.enter_context(tc.tile_pool(name="kv1", bufs=1))
    kT1 = kv1_pool.tile([P, QC, NK1], BF16, tag="kT")
    v1 = kv1_pool.tile([P, NK1 // P, H, DH + 1], BF16, tag="v")
    nc.gpsimd.memset(v1[:, :, :, DH:DH + 1], 1.0)
    wkv1 = kv1_pool.tile([P, FC, 2 * DIM], BF16, tag="wkv")
    nc.gpsimd.dma_start(wkv1[:], io["a1_wkv"].rearrange("(c p) d -> p c d", p=P))
    pstA_es = ExitStack()
    pstA = pstA_es.enter_context(tc.tile_pool(name="pstA", bufs=4, space="PSUM"))
    ctx_src = ("dram", io["context"])
    with tc.tile_pool(name="cn", bufs=2) as cn_pool, \
         tc.tile_pool(name="pskv", bufs=2, space="PSUM") as pskv:
        for kb in range(4):
            cn = cn_pool.tile([P, FC, 512], BF16, tag="cn")
            if kb == 0:
                ln_run(ctx_src, [0, 1], cn, pstA)
                ln_run(ctx_src, [2, 3], cn, pstA, dst_col_off=256)
            else:
                ln_run(ctx_src, list(range(kb * 4, kb * 4 + 4)), cn, pstA)
            kv_block(cn, 0, kb, wkv1, kT1, v1, pskv)

    # const bias vectors (first needed at the qTp epilogues ~40us in; emitted
    # here so the SP DMA queue serves the context x-tiles first)
    bq1 = feat_major_vec("fold_bq1", DIM)     # (ln_b @ a1_wq) * SCALE
    bq2 = feat_major_vec("fold_bq2", DIM)
    bp1 = feat_major_vec("fold_bp1", D)       # a1_bp + (ln_b @ a1_wv) @ a1_wp
    bp2 = feat_major_vec("fold_bp2", D)
    b1m1 = feat_major_vec("m1_b1", HID)       # host-folded (+ ln_b @ m1_w1)
    b2m1 = feat_major_vec("m1_b2", D)
    b1m2 = feat_major_vec("m2_b1", HID)
    b2m2 = feat_major_vec("m2_b2", D)

    # ---- phase B: LN(query) -> q out + qT; Q projection for layer 1 only
    # (layer 2's projection is deferred past attn1's emission so attention
    # never waits on it through PSUM bank reuse; it fills the mlp1 ramp).
    def q_project(wq, bq, pool, psq):
        qTp = pool.tile([P, QC, NQ], BF16, tag="qTp")
        for qc in range(QC):
            for qhh in range(2):
                ps = psq.tile([P, 512], F32, tag="psq")
                for f in range(FC):
                    nc.tensor.matmul(
                        ps[:], wq[:, f, qc * P:(qc + 1) * P],
                        qT[:, f, qhh * 512:(qhh + 1) * 512],
                        start=(f == 0), stop=(f == FC - 1))
                nc.scalar.activation(
                    qTp[:, qc, qhh * 512:(qhh + 1) * 512], ps[:],
                    AF.Identity, bias=bq[:, qc:qc + 1], scale=SCALE)
        return qTp

    qT_es = ExitStack()
    qT_pool = qT_es.enter_context(tc.tile_pool(name="qT", bufs=1))
    with tc.tile_pool(name="qop", bufs=4) as qo_pool, \
         tc.tile_pool(name="psq", bufs=2, space="PSUM") as psq:
        wbc = qT_pool.tile([P, D], F32, tag="wbc")
        nc.scalar.dma_start(wbc[:], bass.AP(tensor=io["ln_w"].tensor, offset=0,
                                            ap=[[0, P], [1, D]]))
        bbc = qT_pool.tile([P, D], F32, tag="bbc")
        nc.scalar.dma_start(bbc[:], bass.AP(tensor=io["ln_b"].tensor, offset=0,
                                            ap=[[0, P], [1, D]]))
        wqs = []
        for li, wq_name in enumerate(("a1_wq", "a2_wq")):
            wq = qT_pool.tile([P, FC, DIM], BF16, tag="wq", bufs=2,
                              name=f"wq{li}")
            eng = nc.gpsimd if li == 0 else nc.scalar
            eng.dma_start(wq[:], io[wq_name].rearrange("(c p) d -> p c d", p=P))
            wqs.append(wq)
        qT = qT_pool.tile([P, FC, NQ], BF16)
        for g in range(2):
            ln_run(("dram", io["query"]), list(range(g * 4, g * 4 + 4)), qT,
                   pstA, q_out=io["out_q"], qo_pool=qo_pool,
                   dst_col_off=g * 512)
        qTp1 = q_project(wqs[0], bq1, qTp_pool, psq)

    # ---- phase C: attention 1 + projection (+ streamed LN(c1))
    pstA_es.close()   # free LN psum banks before attention claims them

    def ln_half_into(dstT, srcT):
        def cb(qh, psum):
            ln_run(("feat", srcT), list(range(qh * 4, qh * 4 + 4)), dstT,
                   psum, dst_col_off=qh * 512)
        return cb

    c1T = cstream.tile([P, FC, NQ], BF16, tag="ct")
    lnc1 = lnct_pool.tile([P, FC, NQ], BF16, tag="lnc", name="lnc1")

    mha_attn(kT1, v1, NK1, qTp1, "a1_wp", bp1, None, c1T,
             on_half=ln_half_into(lnc1, c1T))

    # deferred layer-2 Q projection (runs during attn1 drain / mlp1 ramp)
    with tc.tile_pool(name="psq2", bufs=2, space="PSUM") as psq2:
        qTp2 = q_project(wqs[1], bq2, qTp2_pool, psq2)
    qT_es.close()
    kv1_es.close()
    qTp_pool.release()

    # ---- MLP1 (+ streamed LN(c2))
    c2T = cstream.tile([P, FC, NQ], BF16, tag="ct")
    lnc2 = lnct_pool.tile([P, FC, NQ], BF16, tag="lnc", name="lnc2")
    mlp(lnc1, "m1_w1", b1m1, "m1_w2", b2m1, c1T, c2T,
        on_half=ln_half_into(lnc2, c2T))

    # ---- MHA2 (keys/values projected from the streamed LN(c2))
    kv2_es = ExitStack()
    kv2_pool = kv2_es.enter_context(tc.tile_pool(name="kv2", bufs=1))
    kT2 = kv2_pool.tile([P, QC, NQ], BF16, tag="kT")
    v2 = kv2_pool.tile([P, NQT, H, DH + 1], BF16, tag="v")
    nc.gpsimd.memset(v2[:, :, :, DH:DH + 1], 1.0)
    wkv2 = kv2_pool.tile([P, FC, 2 * DIM], BF16, tag="wkv")
    nc.sync.dma_start(wkv2[:], io["a2_wkv"].rearrange("(c p) d -> p c d", p=P))

    c3T = cstream.tile([P, FC, NQ], BF16, tag="ct")
    lnc3 = lnct_pool.tile([P, FC, NQ], BF16, tag="lnc", name="lnc3")
    with tc.tile_pool(name="pskv2", bufs=2, space="PSUM") as pskv2_:
        for kb in (0, 1):
            kv_block(lnc2, kb * 512, kb, wkv2, kT2, v2, pskv2_,
                     act_copies=False)

    mha_attn(kT2, v2, NQ, qTp2, "a2_wp", bp2, c2T, c3T,
             on_half=ln_half_into(lnc3, c3T))
    kv2_es.close()

    # ---- MLP2 (+ streamed final output)
    c4T = cstream.tile([P, FC, NQ], BF16, tag="ct")
    with tc.tile_pool(name="otile", bufs=3) as ot_pool:

        def stream_out(qh, psum):
            for t in range(qh * 4, qh * 4 + 4):
                ot = ot_pool.tile([P, D], F32, tag="ot")
                o0 = psum.tile([P, 512], BF16, tag="pso")
                for c in range(4):
                    nc.tensor.transpose(o0[:, c * P:(c + 1) * P],
                                        c4T[:, c, t * P:(t + 1) * P], ident[:])
                o1 = psum.tile([P, 512], BF16, tag="pso")
                for c in range(2):
                    nc.tensor.transpose(o1[:, c * P:(c + 1) * P],
                                        c4T[:, 4 + c, t * P:(t + 1) * P], ident[:])
                nc.vector.tensor_copy(ot[:, 0:512], o0[:])
                nc.scalar.copy(ot[:, 512:D], o1[:, 0:256])
                eng = nc.sync if t % 2 == 0 else nc.scalar
                eng.dma_start(io["out_c"][t * P:(t + 1) * P, :], ot[:])

        mlp(lnc3, "m2_w1", b1m2, "m2_w2", b2m2, c3T, c4T, on_half=stream_out)


_BF16_INPUTS = {"a1_wq", "a1_wkv", "a1_wp", "m1_w1", "m1_w2",
                "a2_wq", "a2_wkv", "a2_wp", "m2_w1", "m2_w2"}

_IN_SHAPES = {
    "query": (NQ, D), "context": (NK1, D),
    "ln_w": (D,), "ln_b": (D,),
    "a1_wq": (D, DIM), "a1_wkv": (D, 2 * DIM), "a1_wp": (DIM, D),
    "m1_w1": (D, HID), "m1_b1": (HID,), "m1_w2": (HID, D), "m1_b2": (D,),
    "a2_wq": (D, DIM), "a2_wkv": (D, 2 * DIM), "a2_wp": (DIM, D),
    "m2_w1": (D, HID), "m2_b1": (HID,), "m2_w2": (HID, D), "m2_b2": (D,),
    "fold_bq1": (DIM,), "fold_bq2": (DIM,),
    "fold_bp1": (D,), "fold_bp2": (D,),
}

_CACHE = {}


def build(replicas=1):
    key = ("nc", replicas)
    if key in _CACHE:
        return _CACHE[key]
    nc = bacc.Bacc("TRN2", target_bir_lowering=False, debug=False,
                   num_devices=N_CORES)
    io = {}
    for name, shape in _IN_SHAPES.items():
        dt = BF16 if name in _BF16_INPUTS else F32
        io[name] = nc.dram_tensor(name, list(shape), dt, kind="ExternalInput").ap()
    io["out_c"] = nc.dram_tensor("out_c", [NQ, D], F32, kind="ExternalOutput").ap()
    io["out_q"] = nc.dram_tensor("out_q", [NQ, D], F32, kind="ExternalOutput").ap()
    with tile.TileContext(nc) as tc:
        for _ in range(replicas):
            with ExitStack() as ctx:
                _emit(nc, tc, ctx, io)
    nc.compile()
    _CACHE[key] = nc
    return nc


def host_fold(inputs):
    """Fold ln_w into consumer weights; fold ln_b-induced bias terms; fold
    the v-bias through the output projection.  All in f64, exact."""
    f = np.float32
    bf = ml_dtypes.bfloat16
    w = np.asarray(inputs["ln_w"], np.float64)
    b = np.asarray(inputs["ln_b"], np.float64)
    out = {}
    for k, v_ in inputs.items():
        if k in _IN_SHAPES:
            out[k] = np.ascontiguousarray(np.asarray(v_, f))
    scaled = {}
    for wn in ("a1_wq", "a1_wkv", "m1_w1", "a2_wq", "a2_wkv", "m2_w1"):
        scaled[wn] = np.asarray(inputs[wn], np.float64) * w[:, None]
        out[wn] = np.ascontiguousarray(scaled[wn].astype(f))
    out["fold_bq1"] = (b @ scaled["a1_wq"] * SCALE).astype(f)
    out["fold_bq2"] = (b @ scaled["a2_wq"] * SCALE).astype(f)
    for li in (1, 2):
        wkv = scaled[f"a{li}_wkv"]
        wp = np.asarray(inputs[f"a{li}_wp"], np.float64)
        bp = np.asarray(inputs[f"a{li}_bp"], np.float64)
        bv = b @ wkv[:, DIM:]
        out[f"fold_bp{li}"] = (bp + bv @ wp).astype(f)
    out["m1_b1"] = (np.asarray(inputs["m1_b1"], np.float64)
                    + b @ scaled["m1_w1"]).astype(f)
    out["m2_b1"] = (np.asarray(inputs["m2_b1"], np.float64)
                    + b @ scaled["m2_w1"]).astype(f)
    for wn in _BF16_INPUTS:
        out[wn] = np.ascontiguousarray(out[wn].astype(bf))
    return out


def kernel(**inputs):
    nc = build()
    folded = host_fold(inputs)
    in_maps = []
    for i in range(N_CORES):
        m = {}
        for name in _IN_SHAPES:
            a = folded[name]
            if name in ("query", "context"):
                a = np.ascontiguousarray(np.asarray(inputs[name], np.float32)[i])
            m[name] = a
        in_maps.append(m)
    res = run_bass_kernel_spmd(nc, in_maps, list(range(N_CORES)))
    c = np.stack([res.results[i]["out_c"] for i in range(N_CORES)])
    q = np.stack([res.results[i]["out_q"] for i in range(N_CORES)])
    return (c, q)
